# revision 1
# baseline (speedup 1.0000x reference)
"""Trainium2 Bass kernel for batched GCN message passing (nn_MLN_GCN).

Math: per graph b (B=1024 total, data-parallel over 8 cores, 128 graphs/core):
    h0 = x[b,:,None] * embedding                  # [512, 64]
    h1 = relu(A @ (h0 @ W1) + b1)
    h2 = relu(A @ (h1 @ W2) + b2)
    logit = A @ (h2 @ W3) + b3                    # [512]
    out = [softmax(logit[:10]), sigmoid(logit[10:])]
where A[c, r] = sum_{e: col_e=c, row_e=r} norm_e is the (dense 512x512)
normalized adjacency shared by every graph.

Key identities used on device:
  - h0 @ W1 == x[b,:,None] * (embedding @ W1)  -> no matmul for layer-1 transform
  - aggregation done as out[(b,f), n_out] = sum_k y[k,(b,f)] * A_T[k,n_out]
    (activations as the stationary PE operand), transform done as
    out[n, (b,o)] = sum_k h[k(b,f), n] * Wblk[k, (b,o)] with Wblk = diag(W, W)
    for a 2-graph pair -> layouts alternate node-major/feat-major with zero
    transposes in the main loop.
All matmuls in bf16 (fp32 PSUM accumulation); validated end-to-end rel err
~4e-5 against the fp32 reference (softmax/sigmoid compress the error).
"""

import sys

import numpy as np

for _p in ("/opt/trn_rl_repo",):
    if _p not in sys.path:
        sys.path.append(_p)

B, NUM, D, H, E, MAIN = 1024, 512, 64, 64, 4096, 10
NCORES = 8
BC = B // NCORES        # graphs per core
NPAIR = BC // 2         # 2-graph pairs per core
NCH = NUM // 128        # node chunks of 128

_CACHE = {}


def _build():
    """Build (once) the Bass module implementing one core's worth of work."""
    if "nc" in _CACHE:
        return _CACHE["nc"]

    import concourse.bacc as bacc
    import concourse.mybir as mybir
    from concourse import tile
    from concourse.masks import make_identity

    fp32 = mybir.dt.float32
    bf16 = mybir.dt.bfloat16
    AF = mybir.ActivationFunctionType
    AX = mybir.AxisListType

    nc = bacc.Bacc("TRN2", target_bir_lowering=False, debug=False)

    xt_d = nc.dram_tensor("xt", (NUM, BC), fp32, kind="ExternalInput")
    ew1_d = nc.dram_tensor("ew1", (NUM, H), fp32, kind="ExternalInput")
    at_d = nc.dram_tensor("a_t", (NUM, NUM), bf16, kind="ExternalInput")
    w2_d = nc.dram_tensor("w2blk", (128, 128), bf16, kind="ExternalInput")
    w3_d = nc.dram_tensor("w3blk", (128, 2), bf16, kind="ExternalInput")
    b1_d = nc.dram_tensor("b1blk", (128, 1), fp32, kind="ExternalInput")
    b2_d = nc.dram_tensor("b2blk", (128, 1), fp32, kind="ExternalInput")
    b3_d = nc.dram_tensor("b3rep", (128, 1), fp32, kind="ExternalInput")
    out_d = nc.dram_tensor("out", (BC, NUM), fp32, kind="ExternalOutput")

    with tile.TileContext(nc) as tc:
        from contextlib import ExitStack

        with ExitStack() as ctx:
            const = ctx.enter_context(tc.tile_pool(name="const", bufs=1))
            y1p = ctx.enter_context(tc.tile_pool(name="y1", bufs=4))
            z1p = ctx.enter_context(tc.tile_pool(name="z1", bufs=4))
            y2p = ctx.enter_context(tc.tile_pool(name="y2", bufs=4))
            z2p = ctx.enter_context(tc.tile_pool(name="z2", bufs=4))
            t3p = ctx.enter_context(tc.tile_pool(name="t3", bufs=4))
            psA = ctx.enter_context(tc.tile_pool(name="psA", bufs=2, space="PSUM"))
            psT = ctx.enter_context(tc.tile_pool(name="psT", bufs=2, space="PSUM"))
            ps3 = ctx.enter_context(tc.tile_pool(name="ps3", bufs=2, space="PSUM"))

            xt = const.tile([128, NCH, BC], fp32)
            ew1 = const.tile([128, NCH, H], fp32)
            at = const.tile([128, NCH, NUM], bf16)
            w2 = const.tile([128, 128], bf16)
            w3 = const.tile([128, 2], bf16)
            b1t = const.tile([128, 1], fp32)
            b2t = const.tile([128, 1], fp32)
            b3t = const.tile([128, 1], fp32)
            ident = const.tile([128, 128], bf16)
            y3all = const.tile([128, NUM], bf16)    # [b, n] collected over pairs
            y3t = const.tile([128, NCH, 128], bf16)  # [n, c, b] after transpose
            outsb = const.tile([128, NUM], fp32)
            mx = const.tile([128, 1], fp32)
            nmx = const.tile([128, 1], fp32)
            ssum = const.tile([128, 1], fp32)
            rcp = const.tile([128, 1], fp32)
            es = const.tile([128, MAIN], fp32)

            make_identity(nc, ident[:])

            nc.sync.dma_start(xt[:], xt_d.ap().rearrange("(c p) b -> p c b", p=128))
            nc.sync.dma_start(ew1[:], ew1_d.ap().rearrange("(c p) f -> p c f", p=128))
            for c in range(NCH):
                nc.sync.dma_start(at[:, c, :], at_d.ap()[c * 128:(c + 1) * 128, :])
            nc.sync.dma_start(w2[:], w2_d.ap()[:, :])
            nc.sync.dma_start(w3[:], w3_d.ap()[:, :])
            nc.sync.dma_start(b1t[:], b1_d.ap()[:, :])
            nc.sync.dma_start(b2t[:], b2_d.ap()[:, :])
            nc.sync.dma_start(b3t[:], b3_d.ap()[:, :])

            for g in range(NPAIR):
                # ---- layer 1 transform: y1[n,(b,f)] = x[b,n] * EW1[n,f] ----
                y1 = y1p.tile([128, NCH, 2, H], bf16)
                nc.vector.tensor_mul(
                    y1[:],
                    xt[:, :, 2 * g:2 * g + 2].unsqueeze(3).broadcast_to([128, NCH, 2, H]),
                    ew1[:].unsqueeze(2).broadcast_to([128, NCH, 2, H]),
                )
                # ---- layer 1 aggregation: z1[(b,f), n_out] ----
                z1ps = psA.tile([128, NUM], fp32, tag="psA")
                for k in range(NCH):
                    nc.tensor.matmul(
                        z1ps[:], y1[:, k].rearrange("p a f -> p (a f)"), at[:, k, :],
                        start=(k == 0), stop=(k == NCH - 1),
                    )
                z1 = z1p.tile([128, NUM], bf16)
                nc.scalar.activation(z1[:], z1ps[:], AF.Relu, bias=b1t[:])

                # ---- layer 2 transform: y2[n, (b,o)] ----
                t2ps = psT.tile([128, NCH, 128], fp32, tag="psT")
                for j in range(NCH):
                    nc.tensor.matmul(
                        t2ps[:, j, :], z1[:, j * 128:(j + 1) * 128], w2[:],
                        start=True, stop=True,
                    )
                y2 = y2p.tile([128, NCH, 128], bf16)
                nc.vector.tensor_copy(y2[:], t2ps[:])

                # ---- layer 2 aggregation ----
                z2ps = psA.tile([128, NUM], fp32, tag="psA")
                for k in range(NCH):
                    nc.tensor.matmul(
                        z2ps[:], y2[:, k, :], at[:, k, :],
                        start=(k == 0), stop=(k == NCH - 1),
                    )
                z2 = z2p.tile([128, NUM], bf16)
                nc.scalar.activation(z2[:], z2ps[:], AF.Relu, bias=b2t[:])

                # ---- layer 3 transform: y3[b', n] = sum_(b,f) W3blk h2 ----
                t3ps = ps3.tile([2, NUM], fp32, tag="ps3")
                nc.tensor.matmul(t3ps[:], w3[:], z2[:], start=True, stop=True)
                t3sb = t3p.tile([2, NUM], bf16)
                nc.vector.tensor_copy(t3sb[:], t3ps[:])
                # partition-shifting placement into [b, n] via DMA
                nc.sync.dma_start(y3all[2 * g:2 * g + 2, :], t3sb[:])

            # ---- epilogue: transpose y3all -> [n, b], final aggregation ----
            for c in range(NCH):
                trps = psA.tile([128, 128], bf16, tag="tr")
                nc.tensor.transpose(trps[:], y3all[:, c * 128:(c + 1) * 128], ident[:])
                nc.vector.tensor_copy(y3t[:, c, :], trps[:])

            lps = psA.tile([128, NUM], fp32, tag="psA")
            for c in range(NCH):
                nc.tensor.matmul(
                    lps[:], y3t[:, c, :], at[:, c, :],
                    start=(c == 0), stop=(c == NCH - 1),
                )

            # sigmoid segment (+b3)
            nc.scalar.activation(outsb[:, MAIN:], lps[:, MAIN:], AF.Sigmoid, bias=b3t[:])
            # softmax segment (b3 shift cancels)
            nc.vector.tensor_reduce(mx[:], lps[:, :MAIN], axis=AX.X, op=mybir.AluOpType.max)
            nc.scalar.mul(nmx[:], mx[:], -1.0)
            nc.scalar.activation(es[:], lps[:, :MAIN], AF.Exp, bias=nmx[:], accum_out=ssum[:])
            nc.vector.reciprocal(rcp[:], ssum[:])
            nc.vector.tensor_scalar_mul(outsb[:, :MAIN], es[:], rcp[:])

            nc.sync.dma_start(out_d.ap()[:, :], outsb[:])

    nc.compile()
    _CACHE["nc"] = nc
    return nc


def _prep_inputs(x, embedding, W1, b1, W2, b2, W3, b3, edge_row, edge_col):
    """Host-side prep: shard x over cores, build normalized adjacency + packed weights."""
    import ml_dtypes

    bf16 = ml_dtypes.bfloat16
    x = np.asarray(x, np.float32)
    embedding = np.asarray(embedding, np.float32)
    W1 = np.asarray(W1, np.float32)
    W2 = np.asarray(W2, np.float32)
    W3 = np.asarray(W3, np.float32)
    b1 = np.asarray(b1, np.float32)
    b2 = np.asarray(b2, np.float32)
    b3 = np.asarray(b3, np.float32)
    edge_row = np.asarray(edge_row)
    edge_col = np.asarray(edge_col)

    deg = np.zeros(NUM, np.float32)
    np.add.at(deg, edge_col, np.float32(1.0))
    dinv = np.where(deg > 0, (1.0 / np.sqrt(np.maximum(deg, 1.0))), 0.0).astype(np.float32)
    norm = (dinv[edge_row] * dinv[edge_col]).astype(np.float32)
    A = np.zeros((NUM, NUM), np.float32)
    np.add.at(A, (edge_col, edge_row), norm)
    a_t = np.ascontiguousarray(A.T).astype(bf16)

    ew1 = (embedding @ W1).astype(np.float32)

    w2blk = np.zeros((128, 128), np.float32)
    w2blk[:H, :H] = W2
    w2blk[H:, H:] = W2
    w2blk = w2blk.astype(bf16)
    w3blk = np.zeros((128, 2), np.float32)
    w3blk[:H, 0] = W3[:, 0]
    w3blk[H:, 1] = W3[:, 0]
    w3blk = w3blk.astype(bf16)

    b1blk = np.tile(b1, 2).reshape(128, 1).astype(np.float32)
    b2blk = np.tile(b2, 2).reshape(128, 1).astype(np.float32)
    b3rep = np.full((128, 1), b3[0], np.float32)

    shared = dict(ew1=ew1, a_t=a_t, w2blk=w2blk, w3blk=w3blk,
                  b1blk=b1blk, b2blk=b2blk, b3rep=b3rep)
    in_maps = []
    for c in range(NCORES):
        xt = np.ascontiguousarray(x[c * BC:(c + 1) * BC, :].T)
        in_maps.append(dict(xt=xt, **shared))
    return in_maps


def _run(inputs, trace=False):
    from concourse import bass_utils

    nc = _build()
    in_maps = _prep_inputs(**inputs)
    res = bass_utils.run_bass_kernel_spmd(
        nc, in_maps, core_ids=list(range(NCORES)), trace=trace,
    )
    out = np.concatenate([np.asarray(r["out"], np.float32) for r in res.results], axis=0)
    return out, res


def kernel(**inputs) -> np.ndarray:
    out, _ = _run(inputs, trace=False)
    return out


def kernel_traced(**inputs):
    """Returns (output, BassKernelResults with exec_time_ns/profile)."""
    return _run(inputs, trace=True)



# revision 6
# speedup vs baseline: 1.9218x; 1.9218x over previous
"""Trainium2 Bass kernel for batched GCN message passing (nn_MLN_GCN).

Math per graph b (B=1024, data-parallel over 8 cores, 128 graphs/core):
    h0 = x[b,:,None] * embedding                  # [512, 64]
    h1 = relu(A @ (h0 @ W1) + b1)
    h2 = relu(A @ (h1 @ W2) + b2)
    logit = A @ (h2 @ W3) + b3                    # [512]
    out = [softmax(logit[:10]), sigmoid(logit[10:])]
with A[c, r] = sum_{e: col_e=c, row_e=r} norm_e the dense 512x512 normalized
adjacency shared across the batch.

Device structure (per core, 32 "quads" of 4 graphs = 2 pairs each):
  - layouts alternate feat-major [(b,f), n] / node-major [n, (b,f)] so the
    transform (contract features) and aggregation (contract nodes) matmuls
    chain with zero transposes.
  - layer-1 transform is free: h0 @ W1 == x[b,:,None] * (embedding @ W1).
  - layer-3 uses z2-stationary tiny matmuls so the [n, graph] output lands
    node-major directly (no per-pair DMA, no epilogue transposes).
  - aggregation matmuls optionally run fp8e4 DoubleRow (2 rows/cycle);
    quantization scales are folded into host-side ew1/w2/A copies and
    divided back out inside the relu evacuation's activation scale.
  - emission is software-pipelined: PE order per step i is
    agg1(i), t3(i-2), agg2(i-1), t2(i) so every matmul's scalar/vector
    producer finished at least one step earlier (no head-of-line stalls).
  - ~3.5us of dummy matmuls at t=0 warm the PE HAM clock gate while the
    input DMAs land.
"""

import sys

import numpy as np

for _p in ("/opt/trn_rl_repo",):
    if _p not in sys.path:
        sys.path.append(_p)

B, NUM, D, H, E, MAIN = 1024, 512, 64, 64, 4096, 10
NCORES = 8
BC = B // NCORES        # graphs per core
NQ = BC // 4            # quads (4 graphs = 2 pairs) per core
NCH = NUM // 128        # node chunks of 128

USE_FP8 = True          # fp8e4 DoubleRow aggregation matmuls
SY = 64.0               # activation quantization scale (folded into ew1/w2)
SA = 16.0               # adjacency quantization scale

_CACHE = {}


def _build():
    if "nc" in _CACHE:
        return _CACHE["nc"]

    import concourse.bacc as bacc
    import concourse.mybir as mybir
    from concourse import tile

    fp32 = mybir.dt.float32
    bf16 = mybir.dt.bfloat16
    fp8 = mybir.dt.float8e4
    ydt = fp8 if USE_FP8 else bf16
    AF = mybir.ActivationFunctionType
    AX = mybir.AxisListType
    DR = mybir.MatmulPerfMode.DoubleRow
    sinv = 1.0 / (SY * SA) if USE_FP8 else 1.0

    nc = bacc.Bacc("TRN2", target_bir_lowering=False, debug=False)

    xt_d = nc.dram_tensor("xt", (NUM, BC), fp32, kind="ExternalInput")
    ew1_d = nc.dram_tensor("ew1", (NUM, H), fp32, kind="ExternalInput")
    at_d = nc.dram_tensor("a_t", (NUM, NUM), bf16, kind="ExternalInput")
    if USE_FP8:
        at8_d = nc.dram_tensor("a_t8", (NUM, NUM), fp8, kind="ExternalInput")
    w2_d = nc.dram_tensor("w2blk", (128, 128), bf16, kind="ExternalInput")
    w3_d = nc.dram_tensor("w3blk", (128, 2), bf16, kind="ExternalInput")
    b1_d = nc.dram_tensor("b1blk", (128, 1), fp32, kind="ExternalInput")
    b2_d = nc.dram_tensor("b2blk", (128, 1), fp32, kind="ExternalInput")
    b3_d = nc.dram_tensor("b3rep", (128, 1), fp32, kind="ExternalInput")
    out_d = nc.dram_tensor("out", (BC, NUM), fp32, kind="ExternalOutput")

    with tile.TileContext(nc) as tc:
        from contextlib import ExitStack

        with ExitStack() as ctx:
            const = ctx.enter_context(tc.tile_pool(name="const", bufs=1))
            y1p = ctx.enter_context(tc.tile_pool(name="y1", bufs=4))
            z1p = ctx.enter_context(tc.tile_pool(name="z1", bufs=3))
            y2p = ctx.enter_context(tc.tile_pool(name="y2", bufs=3))
            z2p = ctx.enter_context(tc.tile_pool(name="z2", bufs=3))
            zps = ctx.enter_context(tc.tile_pool(name="zps", bufs=3, space="PSUM"))
            tps = ctx.enter_context(tc.tile_pool(name="tps", bufs=1, space="PSUM"))

            xt = const.tile([128, NCH, BC], fp32)
            ew1 = const.tile([128, NCH, H], fp32)
            at = const.tile([128, NCH, NUM], bf16)
            if USE_FP8:
                at8 = const.tile([128, 2, 2, NUM], fp8)  # [p, cpair, klo/hi, n]
            w2 = const.tile([128, 128], bf16)
            w3 = const.tile([128, 2], bf16)
            b1t = const.tile([128, 1], fp32)
            b2t = const.tile([128, 1], fp32)
            b3t = const.tile([128, 1], fp32)
            y3t = const.tile([128, NCH, BC], bf16)   # [n, chunk, graph]
            outsb = const.tile([128, NUM], fp32)
            warm = const.tile([128, NUM], bf16)
            mx = const.tile([128, 1], fp32)
            nmx = const.tile([128, 1], fp32)
            ssum = const.tile([128, 1], fp32)
            rcp = const.tile([128, 1], fp32)
            es = const.tile([128, MAIN], fp32)

            nc.vector.memset(warm[:], 0)

            nc.sync.dma_start(xt[:], xt_d.ap().rearrange("(c p) b -> p c b", p=128))
            nc.sync.dma_start(ew1[:], ew1_d.ap().rearrange("(c p) f -> p c f", p=128))
            if USE_FP8:
                for cp in range(2):
                    nc.sync.dma_start(
                        at8[:, cp],
                        at8_d.ap()[cp * 256:(cp + 1) * 256, :].rearrange(
                            "(k p) n -> p k n", p=128
                        ),
                    )
            for c in range(NCH):
                nc.sync.dma_start(at[:, c, :], at_d.ap()[c * 128:(c + 1) * 128, :])
            nc.sync.dma_start(w2[:], w2_d.ap()[:, :])
            nc.sync.dma_start(w3[:], w3_d.ap()[:, :])
            nc.sync.dma_start(b1t[:], b1_d.ap()[:, :])
            nc.sync.dma_start(b2t[:], b2_d.ap()[:, :])
            nc.sync.dma_start(b3t[:], b3_d.ap()[:, :])

            # ---- PE warmup: ~3.5us of throwaway matmuls so the HAM clock
            # gate is released by the time real work (and input DMAs) land.
            warmps = tps.tile([128, NUM], fp32, tag="t2")
            for _ in range(9):
                nc.tensor.matmul(warmps[:], warm[:, :128], warm[:], start=True, stop=True)

            def y1_mul(q):
                """y1[n, c, (pair,g), f] = x[g, n] * EW1[n, f] (scaled)."""
                y1 = y1p.tile([128, NCH, 4, H], ydt, tag="y1")
                nc.vector.tensor_mul(
                    y1[:],
                    xt[:, :, 4 * q:4 * q + 4].unsqueeze(3).broadcast_to([128, NCH, 4, H]),
                    ew1[:].unsqueeze(2).broadcast_to([128, NCH, 4, H]),
                )
                return y1

            def agg(ps, ytile, ysel):
                """ps[:, p, :] += A-aggregation for pair p; ysel(p, c|cp) -> lhsT."""
                for p in range(2):
                    if USE_FP8:
                        for cp in range(2):
                            nc.tensor.matmul(
                                ps[:, p, :], ysel(p, cp), at8[:, cp],
                                start=(cp == 0), stop=(cp == 1), perf_mode=DR,
                            )
                    else:
                        for c in range(NCH):
                            nc.tensor.matmul(
                                ps[:, p, :], ysel(p, c), at[:, c, :],
                                start=(c == 0), stop=(c == NCH - 1),
                            )

            def stage_agg1(q, y1):
                z1ps = zps.tile([128, 2, NUM], fp32, tag="z")
                if USE_FP8:
                    agg(z1ps, y1, lambda p, cp: y1[:, 2 * cp:2 * cp + 2, 2 * p:2 * p + 2, :])
                else:
                    agg(z1ps, y1, lambda p, c: y1[:, c, 2 * p:2 * p + 2, :])
                z1 = z1p.tile([128, 2, NUM], bf16, tag="z1")
                nc.scalar.activation(z1[:], z1ps[:], AF.Relu, bias=b1t[:], scale=sinv)
                return z1

            def stage_t2(q, z1):
                t2ps = tps.tile([128, 2, NCH, 128], fp32, tag="t2")
                for p in range(2):
                    for j in range(NCH):
                        nc.tensor.matmul(
                            t2ps[:, p, j, :], z1[:, p, j * 128:(j + 1) * 128], w2[:],
                            start=True, stop=True,
                        )
                y2 = y2p.tile([128, 2, NCH, 128], ydt, tag="y2")
                for p in range(2):
                    nc.vector.tensor_copy(y2[:, p], t2ps[:, p])
                return y2

            def stage_agg2(q, y2):
                z2ps = zps.tile([128, 2, NUM], fp32, tag="z")
                if USE_FP8:
                    agg(z2ps, y2, lambda p, cp: y2[:, p, 2 * cp:2 * cp + 2, :])
                else:
                    agg(z2ps, y2, lambda p, c: y2[:, p, c, :])
                z2 = z2p.tile([128, 2, NUM], bf16, tag="z2")
                nc.scalar.activation(z2[:], z2ps[:], AF.Relu, bias=b2t[:], scale=sinv)
                return z2

            def stage_t3(q, z2):
                t3ps = zps.tile([128, 2, NCH, 2], fp32, tag="z")
                for p in range(2):
                    for j in range(NCH):
                        nc.tensor.matmul(
                            t3ps[:, p, j, :], z2[:, p, j * 128:(j + 1) * 128], w3[:],
                            start=True, stop=True,
                        )
                for p in range(2):
                    nc.scalar.activation(
                        y3t[:, :, 4 * q + 2 * p:4 * q + 2 * p + 2],
                        t3ps[:, p],
                        AF.Copy,
                    )

            # ---- software-pipelined main loop over quads ----
            y1s, z1s, y2s, z2s = {}, {}, {}, {}
            y1s[0] = y1_mul(0)
            if NQ > 1:
                y1s[1] = y1_mul(1)
            for i in range(NQ + 2):
                if i + 2 < NQ:
                    y1s[i + 2] = y1_mul(i + 2)
                if i < NQ:
                    z1s[i] = stage_agg1(i, y1s.pop(i))
                if 0 <= i - 2:
                    stage_t3(i - 2, z2s.pop(i - 2))
                if 0 <= i - 1 < NQ:
                    z2s[i - 1] = stage_agg2(i - 1, y2s.pop(i - 1))
                if i < NQ:
                    y2s[i] = stage_t2(i, z1s.pop(i))

            # ---- epilogue: final aggregation of y3t, then softmax/sigmoid ----
            lps = zps.tile([128, NUM], fp32, tag="z")
            for c in range(NCH):
                nc.tensor.matmul(
                    lps[:], y3t[:, c, :], at[:, c, :],
                    start=(c == 0), stop=(c == NCH - 1),
                )
            nc.scalar.activation(outsb[:, MAIN:], lps[:, MAIN:], AF.Sigmoid, bias=b3t[:])
            nc.vector.tensor_reduce(mx[:], lps[:, :MAIN], axis=AX.X, op=mybir.AluOpType.max)
            nc.scalar.mul(nmx[:], mx[:], -1.0)
            nc.scalar.activation(es[:], lps[:, :MAIN], AF.Exp, bias=nmx[:], accum_out=ssum[:])
            nc.vector.reciprocal(rcp[:], ssum[:])
            nc.vector.tensor_scalar_mul(outsb[:, :MAIN], es[:], rcp[:])

            nc.sync.dma_start(out_d.ap()[:, :], outsb[:])

    nc.compile()
    _CACHE["nc"] = nc
    return nc


def _prep_inputs(x, embedding, W1, b1, W2, b2, W3, b3, edge_row, edge_col):
    """Host prep: shard x over cores, build normalized adjacency + packed weights."""
    import ml_dtypes

    bf16 = ml_dtypes.bfloat16
    x = np.asarray(x, np.float32)
    embedding = np.asarray(embedding, np.float32)
    W1 = np.asarray(W1, np.float32)
    W2 = np.asarray(W2, np.float32)
    W3 = np.asarray(W3, np.float32)
    b1 = np.asarray(b1, np.float32)
    b2 = np.asarray(b2, np.float32)
    b3 = np.asarray(b3, np.float32)
    edge_row = np.asarray(edge_row)
    edge_col = np.asarray(edge_col)

    deg = np.zeros(NUM, np.float32)
    np.add.at(deg, edge_col, np.float32(1.0))
    dinv = np.where(deg > 0, (1.0 / np.sqrt(np.maximum(deg, 1.0))), 0.0).astype(np.float32)
    norm = (dinv[edge_row] * dinv[edge_col]).astype(np.float32)
    A = np.zeros((NUM, NUM), np.float32)
    np.add.at(A, (edge_col, edge_row), norm)
    a_t = np.ascontiguousarray(A.T)

    ew1 = (embedding @ W1).astype(np.float32)

    w2blk = np.zeros((128, 128), np.float32)
    w2blk[:H, :H] = W2
    w2blk[H:, H:] = W2
    w3blk = np.zeros((128, 2), np.float32)
    w3blk[:H, 0] = W3[:, 0]
    w3blk[H:, 1] = W3[:, 0]

    shared = dict(
        a_t=a_t.astype(bf16),
        w3blk=w3blk.astype(bf16),
        b1blk=np.tile(b1, 2).reshape(128, 1).astype(np.float32),
        b2blk=np.tile(b2, 2).reshape(128, 1).astype(np.float32),
        b3rep=np.full((128, 1), b3[0], np.float32),
    )
    if USE_FP8:
        import concourse.mybir as mybir

        fp8np = mybir.dt.np(mybir.dt.float8e4)
        shared["a_t8"] = (a_t * SA).astype(fp8np)
        shared["ew1"] = ew1 * np.float32(SY)
        shared["w2blk"] = (w2blk * SY).astype(bf16)
    else:
        shared["ew1"] = ew1
        shared["w2blk"] = w2blk.astype(bf16)

    in_maps = []
    for c in range(NCORES):
        xt = np.ascontiguousarray(x[c * BC:(c + 1) * BC, :].T)
        in_maps.append(dict(xt=xt, **shared))
    return in_maps


def _run(inputs, trace=False):
    from concourse import bass_utils

    nc = _build()
    in_maps = _prep_inputs(**inputs)
    res = bass_utils.run_bass_kernel_spmd(
        nc, in_maps, core_ids=list(range(NCORES)), trace=trace,
    )
    out = np.concatenate([np.asarray(r["out"], np.float32) for r in res.results], axis=0)
    return out, res


def kernel(**inputs) -> np.ndarray:
    out, _ = _run(inputs, trace=False)
    return out


def kernel_traced(**inputs):
    """Returns (output, BassKernelResults with exec_time_ns/profile)."""
    return _run(inputs, trace=True)


# revision 7
# speedup vs baseline: 2.5573x; 1.3307x over previous
"""Trainium2 Bass kernel for batched GCN message passing (nn_MLN_GCN).

Math per graph b (B=1024, data-parallel over 8 cores, 128 graphs/core):
    h0 = x[b,:,None] * embedding                  # [512, 64]
    h1 = relu(A @ (h0 @ W1) + b1)
    h2 = relu(A @ (h1 @ W2) + b2)
    logit = A @ (h2 @ W3) + b3                    # [512]
    out = [softmax(logit[:10]), sigmoid(logit[10:])]
with A[c, r] = sum_{e: col_e=c, row_e=r} norm_e the dense 512x512 normalized
adjacency shared across the batch.

Device structure (per core, 32 "quads" of 4 graphs = 2 pairs each):
  - layouts alternate feat-major [(b,f), n] / node-major [n, (b,f)] so the
    transform (contract features) and aggregation (contract nodes) matmuls
    chain with zero transposes.
  - layer-1 transform is free: h0 @ W1 == x[b,:,None] * (embedding @ W1).
  - layer-3 uses z2-stationary tiny matmuls so the [n, graph] output lands
    node-major directly (no per-pair DMA, no epilogue transposes).
  - aggregation matmuls run fp8e4 DoubleRow (2 rows/cycle); quantization
    scales fold into host-side ew1/w2/A copies, and the hidden activations
    are kept *scaled* (z~ = 1024*z) so the relu evacuations are pure
    bias+relu ops (the 1/1024 folds into the next layer's weights).
  - PSUM evacuations alternate Scalar/Vector per pair (different banks can
    be read in parallel); the x*EW1 multiply runs on the otherwise idle
    GpSimd engine; per-pair single-bank PSUM tiles on a 5-slot rotation
    keep every WAR dependency at least one pipeline step away.
  - emission is software-pipelined: PE order per step i is
    agg1(i), t3(i-2), agg2(i-1), t2(i) so every matmul's producer finished
    at least one step earlier.
  - ~3.5us of dummy matmuls at t=0 warm the PE HAM clock gate while the
    input DMAs land.
"""

import sys

import numpy as np

for _p in ("/opt/trn_rl_repo",):
    if _p not in sys.path:
        sys.path.append(_p)

B, NUM, D, H, E, MAIN = 1024, 512, 64, 64, 4096, 10
NCORES = 8
BC = B // NCORES        # graphs per core
NQ = BC // 4            # quads (4 graphs = 2 pairs) per core
NCH = NUM // 128        # node chunks of 128

USE_FP8 = True          # fp8e4 DoubleRow aggregation matmuls
Y1_ENGINE = "gpsimd"    # engine for the x*EW1 multiply: "gpsimd" | "vector"
SY = 64.0               # activation quantization scale (folded into ew1/w2)
SA = 16.0               # adjacency quantization scale

_CACHE = {}


def _build():
    key = "nc"
    if key in _CACHE:
        return _CACHE[key]

    import concourse.bacc as bacc
    import concourse.mybir as mybir
    from concourse import tile

    fp32 = mybir.dt.float32
    bf16 = mybir.dt.bfloat16
    fp8 = mybir.dt.float8e4
    ydt = fp8 if USE_FP8 else bf16
    AF = mybir.ActivationFunctionType
    AX = mybir.AxisListType
    ALU = mybir.AluOpType
    DR = mybir.MatmulPerfMode.DoubleRow

    nc = bacc.Bacc("TRN2", target_bir_lowering=False, debug=False)

    xt_d = nc.dram_tensor("xt", (NUM, BC), bf16, kind="ExternalInput")
    ew1_d = nc.dram_tensor("ew1", (NUM, H), bf16, kind="ExternalInput")
    at_d = nc.dram_tensor("a_t", (NUM, NUM), bf16, kind="ExternalInput")
    if USE_FP8:
        at8_d = nc.dram_tensor("a_t8", (NUM, NUM), fp8, kind="ExternalInput")
    w2_d = nc.dram_tensor("w2blk", (128, 128), bf16, kind="ExternalInput")
    w3_d = nc.dram_tensor("w3blk", (128, 2), bf16, kind="ExternalInput")
    b1_d = nc.dram_tensor("b1blk", (128, 1), fp32, kind="ExternalInput")
    b2_d = nc.dram_tensor("b2blk", (128, 1), fp32, kind="ExternalInput")
    b3_d = nc.dram_tensor("b3rep", (128, 1), fp32, kind="ExternalInput")
    out_d = nc.dram_tensor("out", (BC, NUM), fp32, kind="ExternalOutput")

    with tile.TileContext(nc) as tc:
        from contextlib import ExitStack

        with ExitStack() as ctx:
            const = ctx.enter_context(tc.tile_pool(name="const", bufs=1))
            y1p = ctx.enter_context(tc.tile_pool(name="y1", bufs=4))
            z1p = ctx.enter_context(tc.tile_pool(name="z1", bufs=3))
            y2p = ctx.enter_context(tc.tile_pool(name="y2", bufs=3))
            z2p = ctx.enter_context(tc.tile_pool(name="z2", bufs=3))
            zps = ctx.enter_context(tc.tile_pool(name="zps", bufs=5, space="PSUM"))
            tps = ctx.enter_context(tc.tile_pool(name="tps", bufs=2, space="PSUM"))
            t3psp = ctx.enter_context(tc.tile_pool(name="t3ps", bufs=1, space="PSUM"))

            xt = const.tile([128, NCH, BC], bf16)
            ew1 = const.tile([128, NCH, H], bf16)
            at = const.tile([128, NCH, NUM], bf16)
            if USE_FP8:
                at8 = const.tile([128, 2, 2, NUM], fp8)  # [p, cpair, klo/hi, n]
            w2 = const.tile([128, 128], bf16)
            w3 = const.tile([128, 2], bf16)
            b1t = const.tile([128, 1], fp32)
            b2t = const.tile([128, 1], fp32)
            b3t = const.tile([128, 1], fp32)
            y3t = const.tile([128, NCH, BC], bf16)   # [n, chunk, graph]
            outsb = const.tile([128, NUM], fp32)
            warm = const.tile([128, NUM], bf16)
            mx = const.tile([128, 1], fp32)
            nmx = const.tile([128, 1], fp32)
            ssum = const.tile([128, 1], fp32)
            rcp = const.tile([128, 1], fp32)
            es = const.tile([128, MAIN], fp32)

            nc.vector.memset(warm[:], 0)

            nc.sync.dma_start(xt[:], xt_d.ap().rearrange("(c p) b -> p c b", p=128))
            nc.sync.dma_start(ew1[:], ew1_d.ap().rearrange("(c p) f -> p c f", p=128))
            if USE_FP8:
                for cp in range(2):
                    nc.sync.dma_start(
                        at8[:, cp],
                        at8_d.ap()[cp * 256:(cp + 1) * 256, :].rearrange(
                            "(k p) n -> p k n", p=128
                        ),
                    )
            for c in range(NCH):
                nc.sync.dma_start(at[:, c, :], at_d.ap()[c * 128:(c + 1) * 128, :])
            nc.sync.dma_start(w2[:], w2_d.ap()[:, :])
            nc.sync.dma_start(w3[:], w3_d.ap()[:, :])
            nc.sync.dma_start(b1t[:], b1_d.ap()[:, :])
            nc.sync.dma_start(b2t[:], b2_d.ap()[:, :])
            nc.sync.dma_start(b3t[:], b3_d.ap()[:, :])

            # ---- PE warmup: ~3.5us of throwaway matmuls so the HAM clock
            # gate is released by the time real work (and input DMAs) land.
            warmps = tps.tile([128, NUM], fp32, tag="t2")
            for _ in range(9):
                nc.tensor.matmul(warmps[:], warm[:, :128], warm[:], start=True, stop=True)

            y1eng = nc.gpsimd if Y1_ENGINE == "gpsimd" else nc.vector

            def y1_mul(q):
                """y1[n, c, (pair,g), f] = x[g, n] * EW1[n, f] (scaled by SY)."""
                y1 = y1p.tile([128, NCH, 4, H], ydt, tag="y1")
                y1eng.tensor_mul(
                    y1[:],
                    xt[:, :, 4 * q:4 * q + 4].unsqueeze(3).broadcast_to([128, NCH, 4, H]),
                    ew1[:].unsqueeze(2).broadcast_to([128, NCH, 4, H]),
                )
                return y1

            def agg_pair(ps, ysel):
                """ps[:] = A-aggregation with lhsT slices from ysel."""
                if USE_FP8:
                    for cp in range(2):
                        nc.tensor.matmul(
                            ps[:], ysel(cp), at8[:, cp],
                            start=(cp == 0), stop=(cp == 1), perf_mode=DR,
                        )
                else:
                    for c in range(NCH):
                        nc.tensor.matmul(
                            ps[:], ysel(c), at[:, c, :],
                            start=(c == 0), stop=(c == NCH - 1),
                        )

            def relu_evac(p, zt, ps, bt):
                """z~[:, p] = relu(ps + b~); Scalar for pair 0, Vector for pair 1."""
                if p == 0:
                    nc.scalar.activation(zt[:, p], ps[:], AF.Relu, bias=bt[:])
                else:
                    nc.vector.tensor_scalar(
                        zt[:, p], ps[:], bt[:], 0.0, ALU.add, ALU.max
                    )

            def stage_agg1(q, y1):
                z1 = z1p.tile([128, 2, NUM], bf16, tag="z1")
                for p in range(2):
                    z1ps = zps.tile([128, NUM], fp32, tag="z")
                    if USE_FP8:
                        agg_pair(z1ps, lambda cp: y1[:, 2 * cp:2 * cp + 2, 2 * p:2 * p + 2, :])
                    else:
                        agg_pair(z1ps, lambda c: y1[:, c, 2 * p:2 * p + 2, :])
                    relu_evac(p, z1, z1ps, b1t)
                return z1

            def stage_t2(q, z1):
                """y2[n, pair, c, (g,o)] = z1 @ W2 (psum scaled to SY)."""
                y2 = y2p.tile([128, 2, NCH, 128], ydt, tag="y2")
                for p in range(2):
                    t2ps = tps.tile([128, NCH, 128], fp32, tag="t2")
                    for j in range(NCH):
                        nc.tensor.matmul(
                            t2ps[:, j, :], z1[:, p, j * 128:(j + 1) * 128], w2[:],
                            start=True, stop=True,
                        )
                    if p == 0:
                        nc.scalar.activation(y2[:, p], t2ps[:], AF.Copy)
                    else:
                        nc.vector.tensor_copy(y2[:, p], t2ps[:])
                return y2

            def stage_agg2(q, y2):
                z2 = z2p.tile([128, 2, NUM], bf16, tag="z2")
                for p in range(2):
                    z2ps = zps.tile([128, NUM], fp32, tag="z")
                    if USE_FP8:
                        agg_pair(z2ps, lambda cp: y2[:, p, 2 * cp:2 * cp + 2, :])
                    else:
                        agg_pair(z2ps, lambda c: y2[:, p, c, :])
                    relu_evac(p, z2, z2ps, b2t)
                return z2

            def stage_t3(q, z2):
                t3ps = t3psp.tile([128, 2, NCH, 2], fp32, tag="t3")
                for p in range(2):
                    for j in range(NCH):
                        nc.tensor.matmul(
                            t3ps[:, p, j, :], z2[:, p, j * 128:(j + 1) * 128], w3[:],
                            start=True, stop=True,
                        )
                for p in range(2):
                    nc.scalar.activation(
                        y3t[:, :, 4 * q + 2 * p:4 * q + 2 * p + 2],
                        t3ps[:, p],
                        AF.Copy,
                    )

            # ---- software-pipelined main loop over quads ----
            y1s, z1s, y2s, z2s = {}, {}, {}, {}
            y1s[0] = y1_mul(0)
            if NQ > 1:
                y1s[1] = y1_mul(1)
            for i in range(NQ + 2):
                if i + 2 < NQ:
                    y1s[i + 2] = y1_mul(i + 2)
                if i < NQ:
                    z1s[i] = stage_agg1(i, y1s.pop(i))
                if 0 <= i - 2:
                    stage_t3(i - 2, z2s.pop(i - 2))
                if 0 <= i - 1 < NQ:
                    z2s[i - 1] = stage_agg2(i - 1, y2s.pop(i - 1))
                if i < NQ:
                    y2s[i] = stage_t2(i, z1s.pop(i))

            # ---- epilogue: final aggregation of y3t, then softmax/sigmoid ----
            lps = zps.tile([128, NUM], fp32, tag="z")
            for c in range(NCH):
                nc.tensor.matmul(
                    lps[:], y3t[:, c, :], at[:, c, :],
                    start=(c == 0), stop=(c == NCH - 1),
                )
            nc.scalar.activation(outsb[:, MAIN:], lps[:, MAIN:], AF.Sigmoid, bias=b3t[:])
            nc.vector.tensor_reduce(mx[:], lps[:, :MAIN], axis=AX.X, op=mybir.AluOpType.max)
            nc.scalar.mul(nmx[:], mx[:], -1.0)
            nc.scalar.activation(es[:], lps[:, :MAIN], AF.Exp, bias=nmx[:], accum_out=ssum[:])
            nc.vector.reciprocal(rcp[:], ssum[:])
            nc.vector.tensor_scalar_mul(outsb[:, :MAIN], es[:], rcp[:])

            nc.sync.dma_start(out_d.ap()[:, :], outsb[:])

    nc.compile()
    _CACHE[key] = nc
    return nc


def _prep_inputs(x, embedding, W1, b1, W2, b2, W3, b3, edge_row, edge_col):
    """Host prep: shard x over cores, build normalized adjacency + packed weights."""
    import ml_dtypes

    bf16 = ml_dtypes.bfloat16
    x = np.asarray(x, np.float32)
    embedding = np.asarray(embedding, np.float32)
    W1 = np.asarray(W1, np.float32)
    W2 = np.asarray(W2, np.float32)
    W3 = np.asarray(W3, np.float32)
    b1 = np.asarray(b1, np.float32)
    b2 = np.asarray(b2, np.float32)
    b3 = np.asarray(b3, np.float32)
    edge_row = np.asarray(edge_row)
    edge_col = np.asarray(edge_col)

    deg = np.zeros(NUM, np.float32)
    np.add.at(deg, edge_col, np.float32(1.0))
    dinv = np.where(deg > 0, (1.0 / np.sqrt(np.maximum(deg, 1.0))), 0.0).astype(np.float32)
    norm = (dinv[edge_row] * dinv[edge_col]).astype(np.float32)
    A = np.zeros((NUM, NUM), np.float32)
    np.add.at(A, (edge_col, edge_row), norm)
    a_t = np.ascontiguousarray(A.T)

    ew1 = (embedding @ W1).astype(np.float32)

    w2blk = np.zeros((128, 128), np.float32)
    w2blk[:H, :H] = W2
    w2blk[H:, H:] = W2
    w3blk = np.zeros((128, 2), np.float32)
    w3blk[:H, 0] = W3[:, 0]
    w3blk[H:, 1] = W3[:, 0]

    # Scale plumbing: y1/y2 are quantized with an extra SY, the adjacency
    # with SA; the hidden z~ activations stay scaled by S = SY*SA, which the
    # next layer's weight copy divides back out.
    S = SY * SA if USE_FP8 else 1.0
    shared = dict(
        a_t=a_t.astype(bf16),
        ew1=(ew1 * np.float32(SY if USE_FP8 else 1.0)).astype(bf16),
        w2blk=(w2blk * np.float32((SY if USE_FP8 else 1.0) / S)).astype(bf16),
        w3blk=(w3blk * np.float32(1.0 / S)).astype(bf16),
        b1blk=np.tile(b1 * S, 2).reshape(128, 1).astype(np.float32),
        b2blk=np.tile(b2 * S, 2).reshape(128, 1).astype(np.float32),
        b3rep=np.full((128, 1), b3[0], np.float32),
    )
    if USE_FP8:
        import concourse.mybir as mybir

        fp8np = mybir.dt.np(mybir.dt.float8e4)
        shared["a_t8"] = (a_t * SA).astype(fp8np)

    in_maps = []
    for c in range(NCORES):
        xt = np.ascontiguousarray(x[c * BC:(c + 1) * BC, :].T).astype(bf16)
        in_maps.append(dict(xt=xt, **shared))
    return in_maps


def _run(inputs, trace=False):
    from concourse import bass_utils

    nc = _build()
    in_maps = _prep_inputs(**inputs)
    res = bass_utils.run_bass_kernel_spmd(
        nc, in_maps, core_ids=list(range(NCORES)), trace=trace,
    )
    out = np.concatenate([np.asarray(r["out"], np.float32) for r in res.results], axis=0)
    return out, res


def kernel(**inputs) -> np.ndarray:
    out, _ = _run(inputs, trace=False)
    return out


def kernel_traced(**inputs):
    """Returns (output, BassKernelResults with exec_time_ns/profile)."""
    return _run(inputs, trace=True)
